# revision 15
# baseline (speedup 1.0000x reference)
"""Two-layer GAT (GraphAttention) forward on 8 Trainium2 NeuronCores.

Math (per layer, reference semantics):
    h  = x @ w                      [N, U]
    a1 = h @ aw1 ; a2 = h @ aw2     [N, H]
    P[i,j,h]    = exp(adj[i,j] * relu(a1[i,h] + a2[j,h]))
    attn[i,j,h] = P / sum_j P
    out[i,h,:]  = sum_j attn[i,j,h] * h[j,:]   -> concat heads -> activation

Key identity: with adj in {0,1},
    P[i,j] = max(adj[i,j] * e1[i] * e2[j], 1),   e1 = exp(a1), e2 = exp(a2)
and softmax rows are invariant to scaling by em1[i] = exp(-a1[i]):
    P'[j,i] = max(adjT[j,i] * e2[j], em1[i])
so per element: one tensor_scalar mul (DVE 4x) or activation-copy (ACT),
plus one fused tensor_tensor max (DVE 2x) against a broadcast em1 row.
Numerator and denominator both come from one PE matmul vs [1 | h].

v3 structure:
  - adjT host-packed into the SBUF layout -> contiguous chunked DMA.
  - Warmup AllGather of a zero scalar gates the `ones` tile: a cross-core
    barrier that aligns compute across cores before layer 0 (the later
    collectives then see minimal peer skew) at near-zero cost.
  - Two pair-gathers of hfp (heads 0+1 after head 1, heads 2+3 after
    head 3) — few enough to avoid serializing the CC core, early enough
    to hide the first one entirely under heads 2-3.
  - em1 row broadcasts via PE (ones^T @ row -> PSUM -> copy).
  - Layer-1 numerator+denominator fused into one matmul with a [1|hfe]
    2-wide stationary.
  - reciprocal_approx_fast everywhere (18 bits, plenty for 2e-2 tol).
"""

import sys

for _p in ("/opt/trn_rl_repo",):
    if _p not in sys.path:
        sys.path.insert(0, _p)

from contextlib import ExitStack

import ml_dtypes
import numpy as np

import concourse.bacc as bacc
import concourse.mybir as mybir
import concourse.tile as tile
from concourse.bass_utils import run_bass_kernel_spmd

F32 = mybir.dt.float32
BF16 = mybir.dt.bfloat16
BF = ml_dtypes.bfloat16

N = 4096          # nodes
FIN = 128         # input features
U0 = 16           # layer-0 units
H0 = 4            # layer-0 heads
NCORES = 8
R = N // NCORES   # local rows per core (512)
NJT = N // 128    # j tiles (32)
GRP = 4           # j-tiles fused per tensor_tensor max
NGRP = NJT // GRP

# Of each group of 4 B-tiles (B = adjT * e2[j]), how many go to DVE
# (tensor_scalar, 4x mode) vs ACT (activation Copy with per-partition scale).
DVE_B_PER_GRP = 2

_CACHE = {}


def _build():
    nc = bacc.Bacc("TRN2", target_bir_lowering=False, debug=False,
                   num_devices=NCORES)

    # ---- I/O ----
    d_adjP = nc.dram_tensor("adjP", [128, NJT * R], BF16, kind="ExternalInput")
    d_xT = nc.dram_tensor("xT", [FIN, N], BF16, kind="ExternalInput")
    d_xTl = nc.dram_tensor("xTl", [FIN, R], BF16, kind="ExternalInput")
    d_prep = nc.dram_tensor("prep", [FIN, U0 + H0], BF16, kind="ExternalInput")
    d_v1 = nc.dram_tensor("v1", [FIN, H0], BF16, kind="ExternalInput")
    d_w1e = nc.dram_tensor("w1e", [(U0 + 1) * H0, 1], F32, kind="ExternalInput")
    d_aw11 = nc.dram_tensor("aw11", [1, 1], F32, kind="ExternalInput")
    d_aw21 = nc.dram_tensor("aw21", [1, 1], F32, kind="ExternalInput")
    d_y = nc.dram_tensor("y", [1, R], F32, kind="ExternalOutput")

    with ExitStack() as ctx:
        tc = ctx.enter_context(tile.TileContext(nc))
        const = ctx.enter_context(tc.tile_pool(name="const", bufs=1))
        work = ctx.enter_context(tc.tile_pool(name="work", bufs=1))
        bpool = ctx.enter_context(tc.tile_pool(name="bpool", bufs=6))
        ppool = ctx.enter_context(tc.tile_pool(name="ppool", bufs=6))
        dram = ctx.enter_context(tc.tile_pool(name="dram", bufs=1, space="DRAM"))
        pp_misc = ctx.enter_context(tc.tile_pool(name="pp_misc", bufs=2, space="PSUM"))
        pp_hj = ctx.enter_context(tc.tile_pool(name="pp_hj", bufs=2, space="PSUM"))
        pp_acc = ctx.enter_context(tc.tile_pool(name="pp_acc", bufs=2, space="PSUM"))
        pp_bc = ctx.enter_context(tc.tile_pool(name="pp_bc", bufs=1, space="PSUM"))

        # ---- persistent SBUF ----
        sb_adjT = const.tile([128, NJT * R], BF16, tag="adjT")     # 32KB/p
        sb_xT = const.tile([FIN, N], BF16, tag="xT")               # 8KB/p
        sb_xTl = const.tile([FIN, R], BF16, tag="xTl")
        sb_prep = const.tile([FIN, U0 + H0], BF16, tag="prep")     # [w0 | v2]
        sb_v1 = const.tile([FIN, H0], BF16, tag="v1")
        sb_aw11 = const.tile([1, 1], F32, tag="aw11")
        sb_naw11 = const.tile([1, 1], F32, tag="naw11")
        sb_aw21bc = const.tile([128, 1], F32, tag="aw21bc")
        sb_ones = const.tile([1, 128], BF16, tag="ones")
        sb_hj = const.tile([128, NJT * (U0 + 1)], BF16, tag="hj")  # [1 | h] per jt
        sb_e2j = const.tile([128, NJT * H0], F32, tag="e2j")
        sb_em1s = [const.tile([1, R], BF16, tag=f"em1s{h}", name=f"em1s{h}")
                   for h in range(H0)]
        sb_em1bc = [const.tile([128, R], BF16, tag=f"em1bc{h}",
                       name=f"em1bc{h}") for h in range(H0)]
        sb_rec = [const.tile([1, R], F32, tag=f"rec{h}", name=f"rec{h}")
                  for h in range(H0)]
        sb_h1raw = [const.tile([U0 + 1, R], F32, tag=f"h1raw{h}", name=f"h1raw{h}")
                    for h in range(H0)]
        sb_w1h = [const.tile([U0 + 1, 1], F32, tag=f"w1h{h}", name=f"w1h{h}")
                  for h in range(H0)]
        sb_hfp = [const.tile([1, R], F32, tag=f"hfp{h}", name=f"hfp{h}")
                  for h in range(H0)]
        sb_hfpb = [const.tile([1, R], BF16, tag=f"hfpb{h}", name=f"hfpb{h}")
                   for h in range(H0)]
        # harvested gathers, column layout (c two q): c=src core, two=head
        # in pair, q=i//128 within that core's rows
        sb_g01 = const.tile([128, 2 * NJT], BF16, tag="g01")
        sb_g23 = const.tile([128, 2 * NJT], BF16, tag="g23")
        sb_hfeT = const.tile([1, R], F32, tag="hfeT")
        sb_em11 = const.tile([1, R], BF16, tag="em11")
        sb_em1bc1 = const.tile([128, R], BF16, tag="em1bc1")
        sb_hfe1 = const.tile([128, NJT], F32, tag="hfe1")
        sb_e2j1 = const.tile([128, NJT], F32, tag="e2j1")
        sb_hfe1e = const.tile([128, NJT * 2], BF16, tag="hfe1e")
        sb_sigd = work.tile([1, 1], F32, tag="sigd")
        sb_fin = work.tile([1, R], F32, tag="fin")
        sb_fin2 = work.tile([1, R], F32, tag="fin2")

        d_gin01 = dram.tile([2, R], BF16, name="gin01")
        d_gin23 = dram.tile([2, R], BF16, name="gin23")
        d_gout01 = dram.tile([2 * NCORES, R], BF16, addr_space="Shared",
                             name="gout01")
        d_gout23 = dram.tile([2 * NCORES, R], BF16, addr_space="Shared",
                             name="gout23")

        # ---- loads: small tensors first, then xT halves + adjT chunks ----
        # NOTE: no early barrier collective here — cores start with tens of
        # microseconds of launch stagger, and an early sync would force the
        # early cores to eat all of it idle. Late syncs absorb it while
        # compute proceeds.
        nc.sync.dma_start(sb_xTl[:], d_xTl[:])
        nc.sync.dma_start(sb_v1[:], d_v1[:])
        nc.sync.dma_start(sb_prep[:], d_prep[:])
        for h in range(H0):
            nc.sync.dma_start(sb_w1h[h][:],
                              d_w1e[(U0 + 1) * h:(U0 + 1) * (h + 1), :])
        nc.sync.dma_start(sb_aw11[:], d_aw11[:])
        nc.sync.dma_start(sb_aw21bc[:], d_aw21[0:1, 0:1].to_broadcast((128, 1)))
        nc.sync.dma_start(sb_xT[:, 0:2048], d_xT[:, 0:2048])
        nc.sync.dma_start(sb_adjT[:, 0:GRP * R], d_adjP[:, 0:GRP * R])
        nc.sync.dma_start(sb_adjT[:, GRP * R:2 * GRP * R],
                          d_adjP[:, GRP * R:2 * GRP * R])
        nc.sync.dma_start(sb_xT[:, 2048:4096], d_xT[:, 2048:4096])
        for g in range(2, NGRP):
            nc.sync.dma_start(sb_adjT[:, GRP * R * g:GRP * R * (g + 1)],
                              d_adjP[:, GRP * R * g:GRP * R * (g + 1)])

        nc.vector.memset(sb_ones[:], 1.0)
        nc.vector.memset(sb_hj[:], 1.0)
        nc.vector.memset(sb_hfe1e[:], 1.0)
        nc.vector.tensor_scalar_mul(sb_naw11[:], sb_aw11[:], -1.0)

        def bcast_row(dst_bf16, src_row):
            """dst[128, R] = broadcast src_row[1, R] via PE outer product."""
            ps_bc = pp_bc.tile([128, R], F32, tag="bc", name="ps_bc")
            nc.tensor.matmul(ps_bc[:], sb_ones[:], src_row, start=True,
                             stop=True)
            nc.vector.tensor_copy(dst_bf16, ps_bc[:])

        def head_prep(h):
            """a1 -> em1s -> em1bc for head h (PE broadcast, no DRAM trip)."""
            ps_a1 = pp_misc.tile([1, R], F32, tag="misc", name="ps_a1")
            nc.tensor.matmul(ps_a1[:], sb_v1[:, h:h + 1], sb_xTl[:],
                             start=True, stop=True)
            nc.scalar.activation(sb_em1s[h][:], ps_a1[:],
                                 mybir.ActivationFunctionType.Exp, scale=-1.0)
            bcast_row(sb_em1bc[h][:], sb_em1s[h][:])

        head_prep(0)

        # ---- prep: h/e2 per j-tile (interleaved with head-0 groups) ----
        W = U0 + H0

        def prep_chunk(q4):
            ps4 = pp_hj.tile([128, 4 * W], F32, tag="hj", name="ps4")
            for q in range(4):
                jt = 4 * q4 + q
                nc.tensor.matmul(ps4[:, W * q:W * (q + 1)],
                                 sb_xT[:, 128 * jt:128 * (jt + 1)],
                                 sb_prep[:], start=True, stop=True)
            hjv = sb_hj[:, 4 * (U0 + 1) * q4:4 * (U0 + 1) * (q4 + 1)].rearrange(
                "p (q c) -> p q c", q=4)[:, :, 1:U0 + 1]
            psv = ps4[:].rearrange("p (q c) -> p q c", q=4)[:, :, 0:U0]
            nc.vector.tensor_copy(hjv, psv)
            e2v = sb_e2j[:, 4 * H0 * q4:4 * H0 * (q4 + 1)].rearrange(
                "p (q c) -> p q c", q=4)
            pse = ps4[:].rearrange("p (q c) -> p q c", q=4)[:, :, U0:U0 + H0]
            nc.scalar.activation(e2v, pse, mybir.ActivationFunctionType.Exp)

        def harvest(dst, d_gout2):
            # dst[p, (c two q)] = gout[(c two), (q p)]; the source view
            # [(c two q), p] is uniformly strided (stride 128), so this
            # routes through the fast xbar-transpose DMA path.
            src = d_gout2[:].rearrange("(c two) (q p) -> (c two q) p", two=2,
                                       p=128)
            nc.sync.dma_start_transpose(dst[:], src)

        # ---- layer 0 main ----
        for h in range(H0):
            ps_acc = pp_acc.tile([U0 + 1, R], F32, tag="acc")
            with nc.named_scope(f"h{h}"):
                for g in range(NGRP):
                    if h == 0:
                        prep_chunk(g)
                    if g == 1 and h + 1 < H0:
                        head_prep(h + 1)
                    t_B = bpool.tile([128, GRP * R], BF16, tag="B")
                    for k in list(range(DVE_B_PER_GRP, GRP)) + list(range(DVE_B_PER_GRP)):
                        jt = GRP * g + k
                        dst = t_B[:, R * k:R * (k + 1)]
                        src = sb_adjT[:, R * jt:R * (jt + 1)]
                        sc = sb_e2j[:, H0 * jt + h:H0 * jt + h + 1]
                        if k < DVE_B_PER_GRP:
                            nc.vector.tensor_scalar_mul(dst, src, sc)
                        else:
                            nc.scalar.mul(dst, src, sc)
                    t_P = ppool.tile([128, GRP * R], BF16, tag="P")
                    nc.vector.tensor_tensor(
                        t_P[:].rearrange("p (g i) -> p g i", g=GRP),
                        t_B[:].rearrange("p (g i) -> p g i", g=GRP),
                        sb_em1bc[h][:, None, :].to_broadcast((128, GRP, R)),
                        mybir.AluOpType.max)
                    for k in range(GRP):
                        jt = GRP * g + k
                        nc.tensor.matmul(
                            ps_acc[:],
                            sb_hj[:, (U0 + 1) * jt:(U0 + 1) * (jt + 1)],
                            t_P[:, R * k:R * (k + 1)],
                            start=(jt == 0), stop=(jt == NJT - 1))
            with nc.named_scope(f"t{h}"):
                nc.scalar.activation(sb_h1raw[h][:], ps_acc[:],
                                     mybir.ActivationFunctionType.Relu)
                nc.vector.reciprocal_approx_fast(sb_rec[h][:],
                                                 sb_h1raw[h][0:1, :])
                ps_s = pp_misc.tile([1, R], F32, tag="misc", name="ps_s")
                nc.tensor.matmul(ps_s[:], sb_w1h[h][:], sb_h1raw[h][:],
                                 start=True, stop=True)
                nc.vector.tensor_mul(sb_hfp[h][:], ps_s[:], sb_rec[h][:])
                nc.scalar.copy(sb_hfpb[h][:], sb_hfp[h][:])
                if h == 1:
                    nc.vector.tensor_add(sb_hfeT[:], sb_hfp[0][:], sb_hfp[1][:])
                    nc.sync.dma_start(d_gin01[0:1, :], sb_hfpb[0][:])
                    nc.sync.dma_start(d_gin01[1:2, :], sb_hfpb[1][:])
                    nc.gpsimd.collective_compute(
                        "AllGather", mybir.AluOpType.bypass,
                        replica_groups=[list(range(NCORES))],
                        ins=[d_gin01[:].opt()], outs=[d_gout01[:].opt()])
                elif h == 2:
                    nc.vector.tensor_add(sb_hfeT[:], sb_hfeT[:], sb_hfp[2][:])
                elif h == 3:
                    nc.vector.tensor_add(sb_hfeT[:], sb_hfeT[:], sb_hfp[3][:])
                    # harvest the (long-finished) first gather, queue second
                    harvest(sb_g01, d_gout01)
                    nc.sync.dma_start(d_gin23[0:1, :], sb_hfpb[2][:])
                    nc.sync.dma_start(d_gin23[1:2, :], sb_hfpb[3][:])
                    nc.gpsimd.collective_compute(
                        "AllGather", mybir.AluOpType.bypass,
                        replica_groups=[list(range(NCORES))],
                        ins=[d_gin23[:].opt()], outs=[d_gout23[:].opt()])

        # ---- layer 1 prep ----
        with nc.named_scope("l1prep"):
            nc.scalar.activation(sb_em11[:], sb_hfeT[:],
                                 mybir.ActivationFunctionType.Exp,
                                 scale=sb_naw11[:])
            bcast_row(sb_em1bc1[:], sb_em11[:])
            # columns are (c two q): jt = 4c+q lives at (c, two, q); sum the
            # two heads of each pair into hfe1[p, (c q)]
            hfe1v = sb_hfe1[:].rearrange("p (c q) -> p c q", q=4)
            g01v = sb_g01[:].rearrange("p (c two q) -> p c two q", two=2, q=4)
            nc.vector.tensor_add(hfe1v, g01v[:, :, 0, :], g01v[:, :, 1, :])
            harvest(sb_g23, d_gout23)
            g23v = sb_g23[:].rearrange("p (c two q) -> p c two q", two=2, q=4)
            nc.vector.tensor_add(hfe1v, hfe1v, g23v[:, :, 0, :])
            nc.vector.tensor_add(hfe1v, hfe1v, g23v[:, :, 1, :])
            nc.scalar.activation(sb_e2j1[:], sb_hfe1[:],
                                 mybir.ActivationFunctionType.Exp,
                                 scale=sb_aw21bc[:])
            # stationary pairs per jt are [1 | hfe]: denom lands on PSUM
            # partition 0 (readable by the custom recip op), numer on row 1.
            nc.vector.tensor_copy(
                sb_hfe1e[:].rearrange("p (t two) -> p t two", two=2)[:, :, 1:2],
                sb_hfe1[:][:, :, None])
            # warm the sigmoid table set while the PE/DVE chew on layer 1
            nc.scalar.activation(sb_sigd[:], sb_sigd[:],
                                 mybir.ActivationFunctionType.Sigmoid)

        # ---- layer 1 main: fused [1|hfe] stationary, one matmul per jt ----
        ps_l1 = pp_acc.tile([2, R], F32, tag="l1", name="ps_l1", bufs=1)
        with nc.named_scope("l1"):
            for g in range(NGRP):
                t_B = bpool.tile([128, GRP * R], BF16, tag="B")
                for k in list(range(DVE_B_PER_GRP, GRP)) + list(range(DVE_B_PER_GRP)):
                    jt = GRP * g + k
                    dst = t_B[:, R * k:R * (k + 1)]
                    src = sb_adjT[:, R * jt:R * (jt + 1)]
                    sc = sb_e2j1[:, jt:jt + 1]
                    if k < DVE_B_PER_GRP:
                        nc.vector.tensor_scalar_mul(dst, src, sc)
                    else:
                        nc.scalar.mul(dst, src, sc)
                t_P = ppool.tile([128, GRP * R], BF16, tag="P")
                nc.vector.tensor_tensor(
                    t_P[:].rearrange("p (g i) -> p g i", g=GRP),
                    t_B[:].rearrange("p (g i) -> p g i", g=GRP),
                    sb_em1bc1[:, None, :].to_broadcast((128, GRP, R)),
                    mybir.AluOpType.max)
                for k in range(GRP):
                    jt = GRP * g + k
                    nc.tensor.matmul(
                        ps_l1[:], sb_hfe1e[:, 2 * jt:2 * jt + 2],
                        t_P[:, R * k:R * (k + 1)],
                        start=(jt == 0), stop=(jt == NJT - 1))

        # ---- final: sigmoid(numer/denom) ----
        # ps_l1 row 0 = denom, row 1 = numer. DVE lanes cannot cross
        # partitions, so the numer row hops to partition 0 via a tiny DMA
        # while the reciprocal of the denom runs.
        sb_nd = work.tile([2, R], F32, tag="nd")
        sb_num = work.tile([1, R], F32, tag="num")
        with nc.named_scope("fin"):
            nc.vector.tensor_copy(sb_nd[:], ps_l1[:])
            nc.sync.dma_start(sb_num[:], sb_nd[1:2, :])
            nc.vector.reciprocal_approx_fast(sb_fin[:], sb_nd[0:1, :])
            nc.vector.tensor_mul(sb_fin2[:], sb_num[:], sb_fin[:])
            nc.scalar.activation(sb_fin[:], sb_fin2[:],
                                 mybir.ActivationFunctionType.Sigmoid)
            nc.sync.dma_start(d_y[:], sb_fin[:])

    nc.compile()
    return nc


def _prep_inputs(x, adj, w0, aw1_0, aw2_0, w1, aw1_1, aw2_1):
    x = np.asarray(x, np.float32)
    adj = np.asarray(adj, np.float32)
    xT = np.ascontiguousarray(x.T.astype(BF))
    adjT = np.asarray(adj.T, BF)                        # [N, N], exact 0/1
    w0f = np.asarray(w0, np.float32)
    v1 = np.ascontiguousarray((w0f @ np.asarray(aw1_0, np.float32)).astype(BF))
    v2 = (w0f @ np.asarray(aw2_0, np.float32)).astype(BF)
    prep = np.ascontiguousarray(
        np.concatenate([w0f.astype(BF), v2], axis=1))
    w1f = np.asarray(w1, np.float32).reshape(H0, U0)
    w1e = np.zeros((H0, U0 + 1), np.float32)
    w1e[:, 1:] = w1f
    w1e = np.ascontiguousarray(w1e.reshape((U0 + 1) * H0, 1))
    aw11 = np.asarray(aw1_1, np.float32).reshape(1, 1)
    aw21 = np.asarray(aw2_1, np.float32).reshape(1, 1)
    in_maps = []
    for c in range(NCORES):
        rows = slice(R * c, R * (c + 1))
        # pack adjT[:, rows] into the SBUF layout [128, NJT*R]:
        # sb[p, jt*R + i] = adjT[128*jt + p, rows.start + i]
        adjP = np.ascontiguousarray(
            adjT[:, rows].reshape(NJT, 128, R).transpose(1, 0, 2).reshape(
                128, NJT * R))
        in_maps.append({
            "adjP": adjP,
            "xT": xT,
            "xTl": np.ascontiguousarray(xT[:, rows]),
            "prep": prep, "v1": v1, "w1e": w1e,
            "aw11": aw11, "aw21": aw21,
        })
    return in_maps


def run(inputs, trace=False, trace_cores=None):
    if "nc" not in _CACHE:
        _CACHE["nc"] = _build()
    nc = _CACHE["nc"]
    in_maps = _prep_inputs(**inputs)
    res = run_bass_kernel_spmd(nc, in_maps, list(range(NCORES)), trace=trace,
                               trace_cores=trace_cores)
    y = np.concatenate([res.results[c]["y"][0] for c in range(NCORES)])
    return np.ascontiguousarray(y.astype(np.float32)), res


def kernel(**inputs):
    y, _ = run(inputs)
    return y


# revision 16
# speedup vs baseline: 2.0781x; 2.0781x over previous
"""Two-layer GAT (GraphAttention) forward on 8 Trainium2 NeuronCores.

Math (per layer, reference semantics):
    h  = x @ w                      [N, U]
    a1 = h @ aw1 ; a2 = h @ aw2     [N, H]
    P[i,j,h]    = exp(adj[i,j] * relu(a1[i,h] + a2[j,h]))
    attn[i,j,h] = P / sum_j P
    out[i,h,:]  = sum_j attn[i,j,h] * h[j,:]   -> concat heads -> activation

Key identity: with adj in {0,1},
    P[i,j] = max(adj[i,j] * e1[i] * e2[j], 1),   e1 = exp(a1), e2 = exp(a2)
and softmax rows are invariant to scaling by em1[i] = exp(-a1[i]):
    P'[j,i] = max(adjT[j,i] * e2[j], em1[i])
so per element: one tensor_scalar mul (DVE 4x) or activation-copy (ACT),
plus one fused tensor_tensor max (DVE 2x) against a broadcast em1 row.
Numerator and denominator both come from one PE matmul vs [1 | h].

v3 structure:
  - adjT host-packed into the SBUF layout -> contiguous chunked DMA.
  - Warmup AllGather of a zero scalar gates the `ones` tile: a cross-core
    barrier that aligns compute across cores before layer 0 (the later
    collectives then see minimal peer skew) at near-zero cost.
  - Two pair-gathers of hfp (heads 0+1 after head 1, heads 2+3 after
    head 3) — few enough to avoid serializing the CC core, early enough
    to hide the first one entirely under heads 2-3.
  - em1 row broadcasts via PE (ones^T @ row -> PSUM -> copy).
  - Layer-1 numerator+denominator fused into one matmul with a [1|hfe]
    2-wide stationary.
  - reciprocal_approx_fast everywhere (18 bits, plenty for 2e-2 tol).
"""

import sys

for _p in ("/opt/trn_rl_repo",):
    if _p not in sys.path:
        sys.path.insert(0, _p)

from contextlib import ExitStack

import ml_dtypes
import numpy as np

import concourse.bacc as bacc
import concourse.mybir as mybir
import concourse.tile as tile
from concourse.bass_utils import run_bass_kernel_spmd

F32 = mybir.dt.float32
BF16 = mybir.dt.bfloat16
BF = ml_dtypes.bfloat16

N = 4096          # nodes
FIN = 128         # input features
U0 = 16           # layer-0 units
H0 = 4            # layer-0 heads
NCORES = 8
R = N // NCORES   # local rows per core (512)
NJT = N // 128    # j tiles (32)
GRP = 4           # j-tiles fused per tensor_tensor max
NGRP = NJT // GRP

# Of each group of 4 B-tiles (B = adjT * e2[j]), how many go to DVE
# (tensor_scalar, 4x mode) vs ACT (activation Copy with per-partition scale).
DVE_B_PER_GRP = 2

_CACHE = {}


def _build():
    nc = bacc.Bacc("TRN2", target_bir_lowering=False, debug=False,
                   num_devices=NCORES)

    # ---- I/O ----
    d_adjP = nc.dram_tensor("adjP", [128, NJT * R], BF16, kind="ExternalInput")
    d_xT = nc.dram_tensor("xT", [FIN, N], BF16, kind="ExternalInput")
    d_xTl = nc.dram_tensor("xTl", [FIN, R], BF16, kind="ExternalInput")
    d_prep = nc.dram_tensor("prep", [FIN, U0 + H0], BF16, kind="ExternalInput")
    d_v1 = nc.dram_tensor("v1", [FIN, H0], BF16, kind="ExternalInput")
    d_w1e = nc.dram_tensor("w1e", [(U0 + 1) * H0, 1], F32, kind="ExternalInput")
    d_aw11 = nc.dram_tensor("aw11", [1, 1], F32, kind="ExternalInput")
    d_aw21 = nc.dram_tensor("aw21", [1, 1], F32, kind="ExternalInput")
    d_y = nc.dram_tensor("y", [1, R], F32, kind="ExternalOutput")

    with ExitStack() as ctx:
        tc = ctx.enter_context(tile.TileContext(nc))
        const = ctx.enter_context(tc.tile_pool(name="const", bufs=1))
        work = ctx.enter_context(tc.tile_pool(name="work", bufs=1))
        bpool = ctx.enter_context(tc.tile_pool(name="bpool", bufs=6))
        ppool = ctx.enter_context(tc.tile_pool(name="ppool", bufs=6))
        dram = ctx.enter_context(tc.tile_pool(name="dram", bufs=1, space="DRAM"))
        pp_misc = ctx.enter_context(tc.tile_pool(name="pp_misc", bufs=2, space="PSUM"))
        pp_hj = ctx.enter_context(tc.tile_pool(name="pp_hj", bufs=2, space="PSUM"))
        pp_acc = ctx.enter_context(tc.tile_pool(name="pp_acc", bufs=2, space="PSUM"))
        pp_bc = ctx.enter_context(tc.tile_pool(name="pp_bc", bufs=1, space="PSUM"))

        # ---- persistent SBUF ----
        sb_adjT = const.tile([128, NJT * R], BF16, tag="adjT")     # 32KB/p
        sb_xT = const.tile([FIN, N], BF16, tag="xT")               # 8KB/p
        sb_xTl = const.tile([FIN, R], BF16, tag="xTl")
        sb_prep = const.tile([FIN, U0 + H0], BF16, tag="prep")     # [w0 | v2]
        sb_v1 = const.tile([FIN, H0], BF16, tag="v1")
        sb_aw11 = const.tile([1, 1], F32, tag="aw11")
        sb_naw11 = const.tile([1, 1], F32, tag="naw11")
        sb_aw21bc = const.tile([128, 1], F32, tag="aw21bc")
        sb_ones = const.tile([1, 128], BF16, tag="ones")
        sb_hj = const.tile([128, NJT * (U0 + 1)], BF16, tag="hj")  # [1 | h] per jt
        sb_e2j = const.tile([128, NJT * H0], F32, tag="e2j")
        sb_em1s = [const.tile([1, R], BF16, tag=f"em1s{h}", name=f"em1s{h}")
                   for h in range(H0)]
        sb_em1bc = [const.tile([128, R], BF16, tag=f"em1bc{h}",
                       name=f"em1bc{h}") for h in range(H0)]
        sb_rec = [const.tile([1, R], F32, tag=f"rec{h}", name=f"rec{h}")
                  for h in range(H0)]
        sb_h1raw = [const.tile([U0 + 1, R], F32, tag=f"h1raw{h}", name=f"h1raw{h}")
                    for h in range(H0)]
        sb_w1h = [const.tile([U0 + 1, 1], F32, tag=f"w1h{h}", name=f"w1h{h}")
                  for h in range(H0)]
        sb_hfp = [const.tile([1, R], F32, tag=f"hfp{h}", name=f"hfp{h}")
                  for h in range(H0)]
        sb_hfpb = [const.tile([1, R], BF16, tag=f"hfpb{h}", name=f"hfpb{h}")
                   for h in range(H0)]
        # harvested gathers, column layout (c two q): c=src core, two=head
        # in pair, q=i//128 within that core's rows
        sb_g01 = const.tile([128, 2 * NJT], BF16, tag="g01")
        sb_g23 = const.tile([128, 2 * NJT], BF16, tag="g23")
        sb_hfeT = const.tile([1, R], F32, tag="hfeT")
        sb_em11 = const.tile([1, R], BF16, tag="em11")
        sb_em1bc1 = const.tile([128, R], BF16, tag="em1bc1")
        sb_hfe1 = const.tile([128, NJT], F32, tag="hfe1")
        sb_e2j1 = const.tile([128, NJT], F32, tag="e2j1")
        sb_hfe1e = const.tile([128, NJT * 2], BF16, tag="hfe1e")
        sb_sigd = work.tile([1, 1], F32, tag="sigd")
        sb_fin = work.tile([1, R], F32, tag="fin")
        sb_fin2 = work.tile([1, R], F32, tag="fin2")

        d_gin01 = dram.tile([2, R], BF16, name="gin01")
        d_gin23 = dram.tile([2, R], BF16, name="gin23")
        d_gout01 = dram.tile([2 * NCORES, R], BF16, addr_space="Shared",
                             name="gout01")
        d_gout23 = dram.tile([2 * NCORES, R], BF16, addr_space="Shared",
                             name="gout23")

        # ---- loads: small tensors first, then xT halves + adjT chunks ----
        # NOTE: no early barrier collective here — cores start with tens of
        # microseconds of launch stagger, and an early sync would force the
        # early cores to eat all of it idle. Late syncs absorb it while
        # compute proceeds.
        nc.sync.dma_start(sb_xTl[:], d_xTl[:])
        nc.sync.dma_start(sb_v1[:], d_v1[:])
        nc.sync.dma_start(sb_prep[:], d_prep[:])
        for h in range(H0):
            nc.sync.dma_start(sb_w1h[h][:],
                              d_w1e[(U0 + 1) * h:(U0 + 1) * (h + 1), :])
        nc.sync.dma_start(sb_aw11[:], d_aw11[:])
        nc.sync.dma_start(sb_aw21bc[:], d_aw21[0:1, 0:1].to_broadcast((128, 1)))
        nc.sync.dma_start(sb_xT[:, 0:2048], d_xT[:, 0:2048])
        nc.sync.dma_start(sb_adjT[:, 0:GRP * R], d_adjP[:, 0:GRP * R])
        nc.sync.dma_start(sb_adjT[:, GRP * R:2 * GRP * R],
                          d_adjP[:, GRP * R:2 * GRP * R])
        nc.sync.dma_start(sb_xT[:, 2048:4096], d_xT[:, 2048:4096])
        for g in range(2, NGRP):
            nc.sync.dma_start(sb_adjT[:, GRP * R * g:GRP * R * (g + 1)],
                              d_adjP[:, GRP * R * g:GRP * R * (g + 1)])

        nc.vector.memset(sb_ones[:], 1.0)
        nc.vector.memset(sb_hj[:], 1.0)
        nc.vector.memset(sb_hfe1e[:], 1.0)
        nc.vector.tensor_scalar_mul(sb_naw11[:], sb_aw11[:], -1.0)

        def bcast_row(dst_bf16, src_row):
            """dst[128, R] = broadcast src_row[1, R] via PE outer product."""
            ps_bc = pp_bc.tile([128, R], F32, tag="bc", name="ps_bc")
            nc.tensor.matmul(ps_bc[:], sb_ones[:], src_row, start=True,
                             stop=True)
            nc.vector.tensor_copy(dst_bf16, ps_bc[:])

        def head_prep(h):
            """a1 -> em1s -> em1bc for head h (PE broadcast, no DRAM trip)."""
            ps_a1 = pp_misc.tile([1, R], F32, tag="misc", name="ps_a1")
            nc.tensor.matmul(ps_a1[:], sb_v1[:, h:h + 1], sb_xTl[:],
                             start=True, stop=True)
            nc.scalar.activation(sb_em1s[h][:], ps_a1[:],
                                 mybir.ActivationFunctionType.Exp, scale=-1.0)
            bcast_row(sb_em1bc[h][:], sb_em1s[h][:])

        head_prep(0)

        # ---- prep: h/e2 per j-tile (interleaved with head-0 groups) ----
        W = U0 + H0

        def prep_chunk(q4):
            ps4 = pp_hj.tile([128, 4 * W], F32, tag="hj", name="ps4")
            for q in range(4):
                jt = 4 * q4 + q
                nc.tensor.matmul(ps4[:, W * q:W * (q + 1)],
                                 sb_xT[:, 128 * jt:128 * (jt + 1)],
                                 sb_prep[:], start=True, stop=True)
            hjv = sb_hj[:, 4 * (U0 + 1) * q4:4 * (U0 + 1) * (q4 + 1)].rearrange(
                "p (q c) -> p q c", q=4)[:, :, 1:U0 + 1]
            psv = ps4[:].rearrange("p (q c) -> p q c", q=4)[:, :, 0:U0]
            nc.vector.tensor_copy(hjv, psv)
            e2v = sb_e2j[:, 4 * H0 * q4:4 * H0 * (q4 + 1)].rearrange(
                "p (q c) -> p q c", q=4)
            pse = ps4[:].rearrange("p (q c) -> p q c", q=4)[:, :, U0:U0 + H0]
            nc.scalar.activation(e2v, pse, mybir.ActivationFunctionType.Exp)

        def harvest(dst, d_gout2):
            # dst[p, (c two q)] = gout[(c two), (q p)]; the source view
            # [(c two q), p] is uniformly strided (stride 128), so this
            # routes through the fast xbar-transpose DMA path.
            src = d_gout2[:].rearrange("(c two) (q p) -> (c two q) p", two=2,
                                       p=128)
            nc.sync.dma_start_transpose(dst[:], src)

        # ---- layer 0 main ----
        for h in range(H0):
            ps_acc = pp_acc.tile([U0 + 1, R], F32, tag="acc")
            with nc.named_scope(f"h{h}"):
                for g in range(NGRP):
                    if h == 0:
                        prep_chunk(g)
                    if g == 1 and h + 1 < H0:
                        head_prep(h + 1)
                    t_B = bpool.tile([128, GRP * R], BF16, tag="B")
                    for k in list(range(DVE_B_PER_GRP, GRP)) + list(range(DVE_B_PER_GRP)):
                        jt = GRP * g + k
                        dst = t_B[:, R * k:R * (k + 1)]
                        src = sb_adjT[:, R * jt:R * (jt + 1)]
                        sc = sb_e2j[:, H0 * jt + h:H0 * jt + h + 1]
                        if k < DVE_B_PER_GRP:
                            nc.vector.tensor_scalar_mul(dst, src, sc)
                        else:
                            nc.scalar.mul(dst, src, sc)
                    t_P = ppool.tile([128, GRP * R], BF16, tag="P")
                    nc.vector.tensor_tensor(
                        t_P[:].rearrange("p (g i) -> p g i", g=GRP),
                        t_B[:].rearrange("p (g i) -> p g i", g=GRP),
                        sb_em1bc[h][:, None, :].to_broadcast((128, GRP, R)),
                        mybir.AluOpType.max)
                    for k in range(GRP):
                        jt = GRP * g + k
                        nc.tensor.matmul(
                            ps_acc[:],
                            sb_hj[:, (U0 + 1) * jt:(U0 + 1) * (jt + 1)],
                            t_P[:, R * k:R * (k + 1)],
                            start=(jt == 0), stop=(jt == NJT - 1))
            with nc.named_scope(f"t{h}"):
                nc.scalar.activation(sb_h1raw[h][:], ps_acc[:],
                                     mybir.ActivationFunctionType.Relu)
                nc.vector.reciprocal_approx_fast(sb_rec[h][:],
                                                 sb_h1raw[h][0:1, :])
                ps_s = pp_misc.tile([1, R], F32, tag="misc", name="ps_s")
                nc.tensor.matmul(ps_s[:], sb_w1h[h][:], sb_h1raw[h][:],
                                 start=True, stop=True)
                nc.vector.tensor_mul(sb_hfp[h][:], ps_s[:], sb_rec[h][:])
                nc.scalar.copy(sb_hfpb[h][:], sb_hfp[h][:])
                if h == 1:
                    nc.vector.tensor_add(sb_hfeT[:], sb_hfp[0][:], sb_hfp[1][:])
                    nc.sync.dma_start(d_gin01[0:1, :], sb_hfpb[0][:])
                    nc.sync.dma_start(d_gin01[1:2, :], sb_hfpb[1][:])
                    nc.gpsimd.collective_compute(
                        "AllGather", mybir.AluOpType.bypass,
                        replica_groups=[list(range(NCORES))],
                        ins=[d_gin01[:].opt()], outs=[d_gout01[:].opt()])
                elif h == 2:
                    nc.vector.tensor_add(sb_hfeT[:], sb_hfeT[:], sb_hfp[2][:])
                elif h == 3:
                    nc.vector.tensor_add(sb_hfeT[:], sb_hfeT[:], sb_hfp[3][:])
                    # queue our second send FIRST — a harvest of gather 1
                    # here would block the Sync queue on peers' (staggered)
                    # sends and delay our own contribution to gather 2.
                    nc.sync.dma_start(d_gin23[0:1, :], sb_hfpb[2][:])
                    nc.sync.dma_start(d_gin23[1:2, :], sb_hfpb[3][:])
                    nc.gpsimd.collective_compute(
                        "AllGather", mybir.AluOpType.bypass,
                        replica_groups=[list(range(NCORES))],
                        ins=[d_gin23[:].opt()], outs=[d_gout23[:].opt()])
                    harvest(sb_g01, d_gout01)

        # ---- layer 1 prep ----
        with nc.named_scope("l1prep"):
            nc.scalar.activation(sb_em11[:], sb_hfeT[:],
                                 mybir.ActivationFunctionType.Exp,
                                 scale=sb_naw11[:])
            bcast_row(sb_em1bc1[:], sb_em11[:])
            # columns are (c two q): jt = 4c+q lives at (c, two, q); sum the
            # two heads of each pair into hfe1[p, (c q)]
            hfe1v = sb_hfe1[:].rearrange("p (c q) -> p c q", q=4)
            g01v = sb_g01[:].rearrange("p (c two q) -> p c two q", two=2, q=4)
            nc.vector.tensor_add(hfe1v, g01v[:, :, 0, :], g01v[:, :, 1, :])
            harvest(sb_g23, d_gout23)
            g23v = sb_g23[:].rearrange("p (c two q) -> p c two q", two=2, q=4)
            nc.vector.tensor_add(hfe1v, hfe1v, g23v[:, :, 0, :])
            nc.vector.tensor_add(hfe1v, hfe1v, g23v[:, :, 1, :])
            nc.scalar.activation(sb_e2j1[:], sb_hfe1[:],
                                 mybir.ActivationFunctionType.Exp,
                                 scale=sb_aw21bc[:])
            # stationary pairs per jt are [1 | hfe]: denom lands on PSUM
            # partition 0 (readable by the custom recip op), numer on row 1.
            nc.vector.tensor_copy(
                sb_hfe1e[:].rearrange("p (t two) -> p t two", two=2)[:, :, 1:2],
                sb_hfe1[:][:, :, None])
            # warm the sigmoid table set while the PE/DVE chew on layer 1
            nc.scalar.activation(sb_sigd[:], sb_sigd[:],
                                 mybir.ActivationFunctionType.Sigmoid)

        # ---- layer 1 main: fused [1|hfe] stationary, one matmul per jt ----
        ps_l1 = pp_acc.tile([2, R], F32, tag="l1", name="ps_l1", bufs=1)
        with nc.named_scope("l1"):
            for g in range(NGRP):
                t_B = bpool.tile([128, GRP * R], BF16, tag="B")
                for k in list(range(DVE_B_PER_GRP, GRP)) + list(range(DVE_B_PER_GRP)):
                    jt = GRP * g + k
                    dst = t_B[:, R * k:R * (k + 1)]
                    src = sb_adjT[:, R * jt:R * (jt + 1)]
                    sc = sb_e2j1[:, jt:jt + 1]
                    if k < DVE_B_PER_GRP:
                        nc.vector.tensor_scalar_mul(dst, src, sc)
                    else:
                        nc.scalar.mul(dst, src, sc)
                t_P = ppool.tile([128, GRP * R], BF16, tag="P")
                nc.vector.tensor_tensor(
                    t_P[:].rearrange("p (g i) -> p g i", g=GRP),
                    t_B[:].rearrange("p (g i) -> p g i", g=GRP),
                    sb_em1bc1[:, None, :].to_broadcast((128, GRP, R)),
                    mybir.AluOpType.max)
                for k in range(GRP):
                    jt = GRP * g + k
                    nc.tensor.matmul(
                        ps_l1[:], sb_hfe1e[:, 2 * jt:2 * jt + 2],
                        t_P[:, R * k:R * (k + 1)],
                        start=(jt == 0), stop=(jt == NJT - 1))

        # ---- final: sigmoid(numer/denom) ----
        # ps_l1 row 0 = denom, row 1 = numer. DVE lanes cannot cross
        # partitions, so the numer row hops to partition 0 via a tiny DMA
        # while the reciprocal of the denom runs.
        sb_nd = work.tile([2, R], F32, tag="nd")
        sb_num = work.tile([1, R], F32, tag="num")
        with nc.named_scope("fin"):
            nc.vector.tensor_copy(sb_nd[:], ps_l1[:])
            nc.sync.dma_start(sb_num[:], sb_nd[1:2, :])
            nc.vector.reciprocal_approx_fast(sb_fin[:], sb_nd[0:1, :])
            nc.vector.tensor_mul(sb_fin2[:], sb_num[:], sb_fin[:])
            nc.scalar.activation(sb_fin[:], sb_fin2[:],
                                 mybir.ActivationFunctionType.Sigmoid)
            nc.sync.dma_start(d_y[:], sb_fin[:])

    nc.compile()
    return nc


def _prep_inputs(x, adj, w0, aw1_0, aw2_0, w1, aw1_1, aw2_1):
    x = np.asarray(x, np.float32)
    adj = np.asarray(adj, np.float32)
    xT = np.ascontiguousarray(x.T.astype(BF))
    adjT = np.asarray(adj.T, BF)                        # [N, N], exact 0/1
    w0f = np.asarray(w0, np.float32)
    v1 = np.ascontiguousarray((w0f @ np.asarray(aw1_0, np.float32)).astype(BF))
    v2 = (w0f @ np.asarray(aw2_0, np.float32)).astype(BF)
    prep = np.ascontiguousarray(
        np.concatenate([w0f.astype(BF), v2], axis=1))
    w1f = np.asarray(w1, np.float32).reshape(H0, U0)
    w1e = np.zeros((H0, U0 + 1), np.float32)
    w1e[:, 1:] = w1f
    w1e = np.ascontiguousarray(w1e.reshape((U0 + 1) * H0, 1))
    aw11 = np.asarray(aw1_1, np.float32).reshape(1, 1)
    aw21 = np.asarray(aw2_1, np.float32).reshape(1, 1)
    in_maps = []
    for c in range(NCORES):
        rows = slice(R * c, R * (c + 1))
        # pack adjT[:, rows] into the SBUF layout [128, NJT*R]:
        # sb[p, jt*R + i] = adjT[128*jt + p, rows.start + i]
        adjP = np.ascontiguousarray(
            adjT[:, rows].reshape(NJT, 128, R).transpose(1, 0, 2).reshape(
                128, NJT * R))
        in_maps.append({
            "adjP": adjP,
            "xT": xT,
            "xTl": np.ascontiguousarray(xT[:, rows]),
            "prep": prep, "v1": v1, "w1e": w1e,
            "aw11": aw11, "aw21": aw21,
        })
    return in_maps


def run(inputs, trace=False, trace_cores=None):
    if "nc" not in _CACHE:
        _CACHE["nc"] = _build()
    nc = _CACHE["nc"]
    in_maps = _prep_inputs(**inputs)
    res = run_bass_kernel_spmd(nc, in_maps, list(range(NCORES)), trace=trace,
                               trace_cores=trace_cores)
    y = np.concatenate([res.results[c]["y"][0] for c in range(NCORES)])
    return np.ascontiguousarray(y.astype(np.float32)), res


def kernel(**inputs):
    y, _ = run(inputs)
    return y
